# revision 11
# baseline (speedup 1.0000x reference)
"""Cross-attention kernel for 8 Trainium2 NeuronCores.

Sharding: core c => batch b = c//4, head-group g = c%4 (3 of 12 heads, 192 dims).
Each core projects q/k/v for its heads, does softmax attention, and computes a
partial output projection (row-split Wo); host sums the 4 partials per batch.

Structure (v2): the kernel is paced by the ScalarE exp stream (one [128,1024]
exp per (head, key-chunk) ~1.08us each; everything else hides under it).

  - mask compaction on host: only mask==1 key/value positions are shipped,
    zero-padded to a multiple of 128. Padded rows have zeroed v rows and a
    zeroed ones-column so they contribute 0 to numerator and denominator.
  - per-head j-loops, software-pipelined: scores(j) -> exp(j) -> attnV(j-1).
  - scores run ROW-PAIRED on the PE (tile_position (0,0)/(64,0)): the nf=0:512
    half streams on array rows 0-63 while nf=512:1024 streams on rows 64-127
    concurrently (2x).  Operands for heads 0/1 are duplicated into both
    partition halves during projection evacuation (DVE partition-shifted
    copies); head 2 runs serial scores (still fits under the exp pace).
  - attnV keeps the ones-column (M=65) for the softmax denominator Z.
  - attnV accumulator is evacuated to SBUF right after each head so one PSUM
    slot suffices; 1/Z via DVE reciprocal_approx_fast (no Ln -> only one
    activation table load ever), broadcast on GpSimd, normalize muls on DVE.
  - Wo is row-paired per head (wo planes duplicated host-side), run as PE
    fillers inside later head loops, staged/accumulated in SBUF (DVE).
  - remaining projections (k cols>=1024, q/k of head 2, v chunks) are emitted
    as fillers inside the head-0/1 loops so the PE never idles long.
  - DMA: few large ordered transfers on the sync HWDGE ring, sized so each
    tensor lands just before first use; PE warmup matmuls (on junk weights)
    flip the HAM clock gate to 2.4GHz before the real stream begins.
  - f16 output partials (host accumulates in f32 and adds bo).
"""

import numpy as np

import concourse.bass as bass
import concourse.mybir as mybir
import concourse.tile as tile
from concourse import bacc
from concourse.bass_utils import run_bass_kernel_spmd

H = 12
D = 768
HD = 64
NQ = 1024
HL = 3            # heads per core
DC = 6            # 768 / 128 contraction chunks
SCALE = HD ** -0.5

f16 = mybir.dt.float16
f32 = mybir.dt.float32

LAST_EXEC_NS = None
LAST_RESULT = None

_programs = {}


def _build(SP: int):
    NCH = SP // 128
    SPA = min(1024, SP)
    EXPF = mybir.ActivationFunctionType.Exp

    nc = bacc.Bacc("TRN2", target_bir_lowering=False, debug=False, num_devices=8)

    pk = nc.dram_tensor("pk", [128, DC, 576], f16, kind="ExternalInput")
    msk = nc.dram_tensor("msk", [128, NCH], f16, kind="ExternalInput")
    qT = nc.dram_tensor("qT", [128, DC, NQ], f16, kind="ExternalInput")
    kTa = nc.dram_tensor("kTa", [128, DC, SPA], f16, kind="ExternalInput")
    vTa = nc.dram_tensor("vTa", [128, DC, SPA], f16, kind="ExternalInput")
    # late columns, split in two for earlier partial arrival
    SPB = SP - SPA
    SPB1 = min(512, SPB)
    SPB2 = SPB - SPB1
    if SPB:
        kTb1 = nc.dram_tensor("kTb1", [128, DC, SPB1], f16, kind="ExternalInput")
        vTb1 = nc.dram_tensor("vTb1", [128, DC, SPB1], f16, kind="ExternalInput")
        if SPB2:
            kTb2 = nc.dram_tensor("kTb2", [128, DC, SPB2], f16, kind="ExternalInput")
            vTb2 = nc.dram_tensor("vTb2", [128, DC, SPB2], f16, kind="ExternalInput")
    wo = nc.dram_tensor("wo", [128, HL, D], f16, kind="ExternalInput")
    out = nc.dram_tensor("out", [NQ, D], f16, kind="ExternalOutput")

    with tile.TileContext(nc) as tc:
        with (
            tc.tile_pool(name="const", bufs=1) as cp,
            tc.tile_pool(name="nrm", bufs=2) as np_,
            tc.tile_pool(name="expp", bufs=4) as ep,
            tc.tile_pool(name="obuf", bufs=2) as op_,
            tc.tile_pool(name="pssc", bufs=2, space="PSUM") as pssc,
            tc.tile_pool(name="psat", bufs=1, space="PSUM") as psat,
            tc.tile_pool(name="psf", bufs=2, space="PSUM") as psf,
        ):
            # ---------------- persistent tiles
            pk_in = cp.tile([128, DC, 576], f16)
            msk_in = cp.tile([128, NCH], f16)
            qT_in = cp.tile([128, DC, NQ], f16)
            kT_in = cp.tile([128, DC, SP], f16)   # a|b column blocks
            vT_in = cp.tile([128, DC, SP], f16)
            wo_in = cp.tile([128, HL, D], f16)
            qh = cp.tile([128, HL, NQ], f16)      # per-head q, dup'd halves
            kh = cp.tile([128, HL, SP], f16)      # per-head k, dup'd halves
            vaug = cp.tile([128, HL * NCH * 65], f16)
            vaug_r = vaug[:].rearrange("p (h j e) -> p h j e", h=HL, j=NCH)
            a_all = cp.tile([128, HL, NQ], f16)   # normalized attn out, dup'd
            ob = cp.tile([128, NQ // 128, D], f32)  # staged Wo partial sums

            # ---------------- DMA (one HWDGE ring; FIFO => ordered arrival)
            nc.sync.dma_start(pk_in[:], pk.ap())
            nc.sync.dma_start(msk_in[:], msk.ap())
            nc.sync.dma_start(qT_in[:], qT.ap())
            kT_r = kT_in[:]
            vT_r = vT_in[:]
            nc.sync.dma_start(kT_r[:, :, 0:SPA], kTa.ap())
            nc.sync.dma_start(vT_r[:, :, 0:SPA], vTa.ap())
            if SPB:
                nc.sync.dma_start(kT_r[:, :, SPA:SPA + SPB1], kTb1.ap())
                nc.sync.dma_start(vT_r[:, :, SPA:SPA + SPB1], vTb1.ap())
                if SPB2:
                    nc.sync.dma_start(kT_r[:, :, SPA + SPB1:SP], kTb2.ap())
                    nc.sync.dma_start(vT_r[:, :, SPA + SPB1:SP], vTb2.ap())
            nc.sync.dma_start(wo_in[:], wo.ap())

            # ---------------- PE warmup: junk matmuls on pk to engage HAM
            for i in range(40):
                ps = psf.tile([128, 128], f32, tag="f")
                nc.tensor.matmul(ps[:], pk_in[:, 0, 0:128], pk_in[:, 1, 0:128],
                                 start=True, stop=True)

            # mask column of vaug (dep: msk only)
            nc.vector.tensor_copy(
                vaug_r[:, :, :, 64],
                msk_in[:].rearrange("p (u j) -> p u j", u=1)
                .broadcast_to([128, HL, NCH]),
            )

            # ---------------- projection helpers
            def proj01(wcol, src_r, dst, dstw, nf, wf):
                """Project heads 0+1 (M=128) and evac with dup'd halves."""
                ps = psf.tile([128, 512], f32, tag="f")
                for d in range(DC):
                    nc.tensor.matmul(
                        ps[:, 0:wf], pk_in[:, d, wcol:wcol + 128],
                        src_r[:, d, nf:nf + wf],
                        start=(d == 0), stop=(d == DC - 1),
                    )
                nc.vector.tensor_copy(dst[0:64, 0, nf:nf + wf], ps[0:64, 0:wf])
                nc.vector.tensor_copy(dst[64:128, 0, nf:nf + wf], ps[0:64, 0:wf])
                nc.vector.tensor_copy(dst[0:64, 1, nf:nf + wf], ps[64:128, 0:wf])
                nc.vector.tensor_copy(dst[64:128, 1, nf:nf + wf], ps[64:128, 0:wf])

            def proj2_pair(jobs):
                """Head-2 q/k projections as col-tile pairs (M=64).

                jobs: list of (wcol, src_r, dst, nf, wf); consecutive pairs
                run in PE col tiles (0,0)/(0,64) concurrently.
                """
                for i in range(0, len(jobs), 2):
                    pair = jobs[i:i + 2]
                    # separate PSUM tiles (banks): start=True clears has_written
                    # for the whole bank, so interleaved accumulation groups
                    # must not share one.
                    pst = [psf.tile([128, 512], f32, tag="f", name=f"p2_{i}_{t}")
                           for t in range(len(pair))]
                    for d in range(DC):
                        for t, (wcol, src_r, dst, nf, wf) in enumerate(pair):
                            nc.tensor.matmul(
                                pst[t][64 * t:64 * t + 64, 0:wf],
                                pk_in[:, d, wcol:wcol + 64],
                                src_r[:, d, nf:nf + wf],
                                start=(d == 0), stop=(d == DC - 1),
                            )
                    for t, (wcol, src_r, dst, nf, wf) in enumerate(pair):
                        nc.vector.tensor_copy(
                            dst[0:64, 2, nf:nf + wf],
                            pst[t][64 * t:64 * t + 64, 0:wf])

            def proj_v(j):
                ps = psf.tile([128, 192], f32, tag="f")
                for d in range(DC):
                    nc.tensor.matmul(
                        ps[:], vT_r[:, d, j * 128:(j + 1) * 128],
                        pk_in[:, d, 384:576],
                        start=(d == 0), stop=(d == DC - 1),
                    )
                nc.vector.tensor_copy(
                    vaug_r[:, :, j, 0:64],
                    ps[:].rearrange("p (h e) -> p h e", h=HL),
                )

            def wo_pair(h, nt):
                """Wo partial for head h, query-tile nt: row-paired halves."""
                HD2 = D // 2
                pa = psf.tile([128, HD2], f32, tag="f")
                pb = psf.tile([128, HD2], f32, tag="f")
                nc.tensor.matmul(
                    pa[:], a_all[0:64, h, nt * 128:(nt + 1) * 128],
                    wo_in[0:64, h, 0:HD2], start=True, stop=True,
                )
                nc.tensor.matmul(
                    pb[:], a_all[64:128, h, nt * 128:(nt + 1) * 128],
                    wo_in[64:128, h, HD2:D], start=True, stop=True,
                )
                if h == 0:
                    nc.vector.tensor_copy(ob[:, nt, 0:HD2], pa[:])
                    nc.vector.tensor_copy(ob[:, nt, HD2:D], pb[:])
                else:
                    nc.vector.tensor_add(ob[:, nt, 0:HD2], ob[:, nt, 0:HD2], pa[:])
                    nc.vector.tensor_add(ob[:, nt, HD2:D], ob[:, nt, HD2:D], pb[:])

            # ---------------- prologue projections (DMA-paced)
            for nf in range(0, NQ, 512):
                proj01(0, qT_in, qh, NQ, nf, 512)          # q heads 0,1
            for nf in range(0, SPA, 512):
                proj01(192, kT_r, kh, SP, nf, 512)         # k heads 0,1 (cols a)
            for j in range(6):                             # first v chunks
                proj_v(j)

            # fillers emitted inside the attention loops, one per iteration,
            # ordered by consumption deadline (v chunk c -> attnV at h0 iter
            # c+1; k cols 512*m -> scores at h0 iter 4m; head-2 proj -> h2).
            fillers = []
            kb_jobs = []
            for nf in range(SPA, SP, 512):
                wf = min(512, SP - nf)
                kb_jobs.append(
                    lambda nf=nf, wf=wf: proj01(192, kT_r, kh, SP, nf, wf))
            v_jobs = [(lambda j=j: proj_v(j)) for j in range(6, NCH)]
            # interleave: v6 v7 kb0 v8 v9 kb1 v10 v11 kb2 v12..
            while v_jobs or kb_jobs:
                for _ in range(2):
                    if v_jobs:
                        fillers.append(v_jobs.pop(0))
                if kb_jobs:
                    fillers.append(kb_jobs.pop(0))
            # head-2 q/k as ragged col pairs (needed only by h2 loop)
            p2jobs = [(128, qT_in, qh, nf, 512) for nf in range(0, NQ, 512)]
            k2jobs = []
            for nf in range(0, SP, 512):
                k2jobs.append((320, kT_r, kh, nf, min(512, SP - nf)))
            mixed = [p2jobs[0], k2jobs[0], p2jobs[1], k2jobs[1]] + k2jobs[2:]
            for i in range(0, len(mixed), 2):
                fillers.append(lambda i=i: proj2_pair(mixed[i:i + 2]))

            def emit_filler():
                if fillers:
                    fillers.pop(0)()

            # ---------------- attention loops (ScalarE-paced)
            ats = [None] * HL

            for h in range(HL):
                at = psat.tile([65, NQ], f32, tag="at")
                prev = None
                for j in range(NCH):
                    sc = pssc.tile([128, NQ], f32, tag="sc")
                    if h < 2:
                        nc.tensor.matmul(
                            sc[:, 0:512], kh[0:64, h, j * 128:(j + 1) * 128],
                            qh[0:64, h, 0:512], start=True, stop=True,
                        )
                        nc.tensor.matmul(
                            sc[:, 512:1024], kh[64:128, h, j * 128:(j + 1) * 128],
                            qh[64:128, h, 512:1024], start=True, stop=True,
                        )
                    else:
                        for nf in range(0, NQ, 512):
                            nc.tensor.matmul(
                                sc[:, nf:nf + 512], kh[0:64, h, j * 128:(j + 1) * 128],
                                qh[0:64, h, nf:nf + 512], start=True, stop=True,
                            )
                    ex = ep.tile([128, NQ], f16, tag="ex")
                    nc.scalar.activation(ex[:], sc[:], EXPF, scale=SCALE)
                    if prev is not None:
                        pj, pex = prev
                        for nf in range(0, NQ, 512):
                            nc.tensor.matmul(
                                at[:, nf:nf + 512],
                                vaug_r[:, h, pj, :], pex[:, nf:nf + 512],
                                start=(pj == 0), stop=False,
                            )
                    prev = (j, ex)
                    emit_filler()
                    # Wo fillers start at j=4 so the previous head's
                    # normalize chain (recip -> broadcast -> muls) has
                    # finished producing a_all before these hit the PE FIFO.
                    if h == 1 and 4 <= j < 4 + NQ // 128:
                        wo_pair(0, j - 4)
                    elif h == 2 and 4 <= j < 4 + NQ // 128:
                        wo_pair(1, j - 4)
                pj, pex = prev
                for nf in range(0, NQ, 512):
                    nc.tensor.matmul(
                        at[:, nf:nf + 512],
                        vaug_r[:, h, pj, :], pex[:, nf:nf + 512],
                        start=(pj == 0), stop=True,
                    )
                # evacuate accumulator so the single PSUM slot recycles fast
                ats_h = np_.tile([65, NQ], f32, tag="ats")
                ats[h] = ats_h
                nc.vector.tensor_copy(ats[h][:], at[:])

                # normalize: a_h = at[0:64] / Z, dup'd into both halves
                rz = np_.tile([1, NQ], f32, tag="rz")
                nc.vector.reciprocal(rz[:], ats[h][64:65, :])
                rzb = np_.tile([64, NQ], f32, tag="rzb")
                nc.gpsimd.partition_broadcast(rzb[:], rz[:])
                nc.vector.tensor_mul(a_all[0:64, h, :], ats[h][0:64, :], rzb[:])
                nc.vector.tensor_mul(a_all[64:128, h, :], ats[h][0:64, :], rzb[:])

            # ---------------- tail: head-2 Wo + final add + out DMA
            HD2 = D // 2
            for nt in range(NQ // 128):
                pa = psf.tile([128, HD2], f32, tag="f")
                pb = psf.tile([128, HD2], f32, tag="f")
                nc.tensor.matmul(
                    pa[:], a_all[0:64, 2, nt * 128:(nt + 1) * 128],
                    wo_in[0:64, 2, 0:HD2], start=True, stop=True,
                )
                nc.tensor.matmul(
                    pb[:], a_all[64:128, 2, nt * 128:(nt + 1) * 128],
                    wo_in[64:128, 2, HD2:D], start=True, stop=True,
                )
                obf = op_.tile([128, D], f16, tag="obf")
                with nc.allow_low_precision(reason="f16 partial output"):
                    nc.vector.tensor_add(obf[:, 0:HD2], ob[:, nt, 0:HD2], pa[:])
                    nc.vector.tensor_add(obf[:, HD2:D], ob[:, nt, HD2:D], pb[:])
                nc.sync.dma_start(out[nt * 128:(nt + 1) * 128, :], obf[:])

    nc.compile()
    return nc


def _get_program(SP: int):
    if SP not in _programs:
        _programs[SP] = _build(SP)
    return _programs[SP]


def _rearr(x):
    """[768, n] -> [128, 6, n] d-chunk layout, f16 contiguous."""
    return np.ascontiguousarray(
        x.reshape(DC, 128, -1).transpose(1, 0, 2).astype(np.float16))


def kernel(query, key, value, mask, Wq, Wk, Wv, Wo, bo):
    query = np.asarray(query, np.float32)
    key = np.asarray(key, np.float32)
    value = np.asarray(value, np.float32)
    mask = np.asarray(mask, np.float32)
    Wq = np.asarray(Wq, np.float32)
    Wk = np.asarray(Wk, np.float32)
    Wv = np.asarray(Wv, np.float32)
    Wo = np.asarray(Wo, np.float32)
    bo = np.asarray(bo, np.float32)

    B, N, _ = query.shape
    idxs = [np.nonzero(mask[b] > 0.5)[0] for b in range(B)]
    se_max = max(len(i) for i in idxs)
    SP = max(((se_max + 127) // 128) * 128, 128)
    NCH = SP // 128
    SPA = min(1024, SP)
    SPB = SP - SPA
    SPB1 = min(512, SPB)
    SPB2 = SPB - SPB1
    nc = _get_program(SP)

    HWID = HL * HD
    in_maps = []
    for c in range(8):
        b, g = c // 4, c % 4
        hs = g * HWID
        idx = idxs[b]
        ne = len(idx)
        kTc = np.zeros((D, SP), np.float32)
        kTc[:, :ne] = key[b].T[:, idx]
        vTc = np.zeros((D, SP), np.float32)
        vTc[:, :ne] = value[b].T[:, idx]
        # msk[p, j] = 1 iff compacted key index j*128+p is a real key
        sidx = np.arange(SP).reshape(NCH, 128).T
        mvec = (sidx < ne).astype(np.float16)

        # pk: wq|wk|wv columns per d-chunk
        wq_r = _rearr(Wq[hs:hs + HWID, :].T)      # [128, 6, 192]
        wk_r = _rearr(Wk[hs:hs + HWID, :].T)
        wv_r = _rearr(Wv[hs:hs + HWID, :].T)
        pk = np.concatenate([wq_r, wk_r, wv_r], axis=2)  # [128, 6, 576]

        # wo planes: head h rows dup'd into both partition halves
        woc = Wo[:, hs:hs + HWID].T.astype(np.float16)   # [192, 768]
        wop = np.empty((128, HL, D), np.float16)
        for h in range(HL):
            wop[0:64, h, :] = woc[h * 64:(h + 1) * 64, :]
            wop[64:128, h, :] = woc[h * 64:(h + 1) * 64, :]

        kTr = _rearr(kTc)
        vTr = _rearr(vTc)
        m = {
            "pk": np.ascontiguousarray(pk),
            "msk": mvec,
            "qT": _rearr(query[b].T),
            "kTa": np.ascontiguousarray(kTr[:, :, 0:SPA]),
            "vTa": np.ascontiguousarray(vTr[:, :, 0:SPA]),
            "wo": wop,
        }
        if SPB:
            m["kTb1"] = np.ascontiguousarray(kTr[:, :, SPA:SPA + SPB1])
            m["vTb1"] = np.ascontiguousarray(vTr[:, :, SPA:SPA + SPB1])
            if SPB2:
                m["kTb2"] = np.ascontiguousarray(kTr[:, :, SPA + SPB1:SP])
                m["vTb2"] = np.ascontiguousarray(vTr[:, :, SPA + SPB1:SP])
        in_maps.append(m)

    r = run_bass_kernel_spmd(nc, in_maps, list(range(8)))
    global LAST_EXEC_NS, LAST_RESULT
    LAST_EXEC_NS = r.exec_time_ns
    LAST_RESULT = r
    res = r.results
    out = np.zeros((B, N, D), np.float32)
    for b in range(B):
        acc = res[4 * b]["out"].astype(np.float32)
        for g in range(1, 4):
            acc += res[4 * b + g]["out"].astype(np.float32)
        out[b] = acc + bo
    return out
